# revision 6
# baseline (speedup 1.0000x reference)
# kernel.py — Bidirectional masked-GRU-with-predictor on 8 Trainium2 NeuronCores.
#
# Problem (reference.py): B=128, T=1024, H=512
#   per step, per direction:
#     x_in = where(mask, predictor(h), x)            predictor: Linear(H,H)->ReLU->Linear(H,1)->Tanh
#     h    = GRUCell(h, x_in)                        PyTorch gate order (r, z, n)
#   output [B, T, 2H] = concat(fwd hidden states, time-reversed bwd hidden states)
#
# Sharding: 8 cores = 2 directions x 4 batch groups of 32.  All cores run the
# SAME Bass program; per-core data differs (bwd cores get time-reversed x/mask
# and their outputs are flipped back on the host).
#
# v4 schedule notes (vs the v1 baseline):
#   - Not weight-load bound in steady state: contiguous W LDW+MM pairs stream
#     at ~27ns.  But every extra 128-col stationary (bias opener, gi block)
#     costs a ~104ns LDW slot and perturbs the pipeline, so aux matmuls are
#     minimized:
#       * biases via 5 whole-region E4 openers (v1 style; per-j start=True
#         accumulation regions measurably corrupt results, so regions are
#         opened once and everything else accumulates with start=False)
#       * the 12 per-(gate,j) rank-1 gi matmuls are consolidated into 3
#         (one per gate): stationary w_ih_g as [4,128], moving X4 =
#         blockdiag(x_in) [4,128], built per step from 4 tiny ones-matmuls
#         (XT) and one DVE multiply with E4.
#   - PH region is emitted with a k01/k23 phase split so the first matmuls of
#     a step need only the first half of h_new (written in two DVE halves).
#   - GIN opener sits right after W_n so PREN = u_n + GIN is never blocked on
#     late PE work; tanh(n) and the final combine run in half-tiles.

import numpy as np

B, T, H = 128, 1024, 512
NCORES = 8
BL = B // 4          # 32: batch per core (4 groups x 2 directions)
KC = H // 128        # 4 contraction chunks
MC = (3 * H + H) // 128  # 16 output chunks (w_hh 12 + p_w1 4)
U_DEF = 64           # time steps per For_i iteration

_cache = {}


def _build_program(t_steps=T, u_steps=U_DEF, bl=BL, n_cores=NCORES):
    import concourse.bacc as bacc
    import concourse.bass as bass
    import concourse.tile as tile
    from concourse.tile import add_dep_helper
    from concourse import mybir

    f16 = mybir.dt.float16
    f32 = mybir.dt.float32

    nc = bacc.Bacc(
        "TRN2",
        target_bir_lowering=False,
        debug=False,
        enable_asserts=False,
        num_devices=n_cores,
    )

    # ---- DRAM tensors (per-core data; same names on every core) ----
    d_wt = nc.dram_tensor("wt", [128, MC * KC * 128], f16, kind="ExternalInput").ap()
    d_gi4 = nc.dram_tensor("gi4", [4, 3 * 128], f16, kind="ExternalInput").ap()
    d_bc = nc.dram_tensor("bc5", [4, 5 * 128], f16, kind="ExternalInput").ap()
    d_o14 = nc.dram_tensor("o14", [1, 4], f16, kind="ExternalInput").ap()
    d_e4 = nc.dram_tensor("e4", [4, KC * bl], f16, kind="ExternalInput").ap()
    d_pw2 = nc.dram_tensor("pw2t", [128, KC], f16, kind="ExternalInput").ap()
    d_pb2 = nc.dram_tensor("pb2", [1, 1], f32, kind="ExternalInput").ap()
    d_a = nc.dram_tensor("a_arr", [t_steps, bl], f16, kind="ExternalInput").ap()
    d_m = nc.dram_tensor("m_arr", [t_steps, bl], f16, kind="ExternalInput").ap()
    d_out = nc.dram_tensor(
        "outl", [t_steps, 128, KC, bl], f16, kind="ExternalOutput"
    ).ap()

    Tanh = mybir.ActivationFunctionType.Tanh
    Sigmoid = mybir.ActivationFunctionType.Sigmoid
    SUB = mybir.AluOpType.subtract
    MULT = mybir.AluOpType.mult

    HB = KC * bl // 2    # 64: half of the (j,b) free dim

    with tile.TileContext(nc) as tc:
        import contextlib

        with contextlib.ExitStack() as ctx:
            consts = ctx.enter_context(tc.tile_pool(name="consts", bufs=1))
            psum = ctx.enter_context(tc.tile_pool(name="psum", bufs=1, space="PSUM"))
            work = ctx.enter_context(tc.tile_pool(name="work", bufs=2))
            io = ctx.enter_context(tc.tile_pool(name="io", bufs=2))

            # ---- constant preload ----
            WT = consts.tile([128, MC * KC * 128], f16, tag="WT")
            GI4 = consts.tile([4, 3 * 128], f16, tag="GI4")
            BC = consts.tile([4, 5 * 128], f16, tag="BC")
            O14 = consts.tile([1, 4], f16, tag="O14")
            E4 = consts.tile([4, KC * bl], f16, tag="E4")
            PW2 = consts.tile([128, KC], f16, tag="PW2")
            PB2 = consts.tile([1, 1], f32, tag="PB2")
            for dst, src in (
                (WT, d_wt), (GI4, d_gi4), (BC, d_bc), (O14, d_o14),
                (E4, d_e4), (PW2, d_pw2), (PB2, d_pb2),
            ):
                nc.sync.dma_start(out=dst, in_=src)

            # persistent ping-pong hidden state, fp16, [128, (j,b)]
            h0 = consts.tile([128, KC * bl], f16, tag="h0")
            h1 = consts.tile([128, KC * bl], f16, tag="h1")
            nc.vector.memset(h0, 0.0)
            nc.vector.memset(h1, 0.0)
            h_tiles = [h0, h1]

            # persistent PSUM accumulators (single-buffered; readers drain early)
            G_r = psum.tile([128, KC * bl], f32, tag="G_r")
            G_z = psum.tile([128, KC * bl], f32, tag="G_z")
            G_n = psum.tile([128, KC * bl], f32, tag="G_n")
            GIN = psum.tile([128, KC * bl], f32, tag="GIN")
            PHp = psum.tile([128, KC * bl], f32, tag="PH")
            PREN = psum.tile([128, KC * bl], f32, tag="PREN")
            PRD = psum.tile([1, bl], f32, tag="PRD")
            XTp = psum.tile([4, KC * bl], f32, tag="XTp")

            def w_block(m, k):
                bi = m * KC + k
                return WT[:, bi * 128:(bi + 1) * 128]

            def jsl(t, j):
                return t[:, j * bl:(j + 1) * bl]

            def step(u, h_cur, h_new, S2, MB, t_dyn, chain):
                # chain: [last PE instruction] single-element list used to
                # force PE issue order across regions/steps (ordering only).
                def mm(out, lhsT, rhs, start, stop):
                    ins = nc.tensor.matmul(
                        out, lhsT, rhs, start=start, stop=stop,
                        skip_group_check=True,
                    )
                    if chain[0] is not None:
                        add_dep_helper(ins.ins, chain[0].ins, sync=False)
                    chain[0] = ins
                    return ins

                u_sl = slice(u * bl, (u + 1) * bl)

                # ---- PE: PH region (opener + phase k01 / k23) ----
                mm(PHp, BC[:, 384:512], E4, True, False)
                for j in range(KC):
                    for k in (0, 1):
                        mm(jsl(PHp, j), w_block(12 + j, k), jsl(h_cur, k),
                           False, False)
                for j in range(KC):
                    for k in (2, 3):
                        mm(jsl(PHp, j), w_block(12 + j, k), jsl(h_cur, k),
                           False, k == 3)

                relu = work.tile([128, KC * bl], f16, tag="relu")
                nc.vector.tensor_scalar_max(relu, PHp, 0.0)

                # ---- PE: W_r (opener + 16) ----
                mm(G_r, BC[:, 0:128], E4, True, False)
                for j in range(KC):
                    for k in range(KC):
                        mm(jsl(G_r, j), w_block(j, k), jsl(h_cur, k),
                           False, False)

                # ---- PE: PRD (predictor 2nd layer) ----
                for k in range(KC):
                    mm(PRD, PW2[:, k:k + 1], jsl(relu, k), k == 0, k == KC - 1)

                pred = work.tile([1, bl], f16, tag="pred")
                nc.scalar.activation(out=pred, in_=PRD, func=Tanh, bias=PB2[:, :])
                xin = work.tile([1, bl], f16, tag="xin")
                nc.vector.tensor_mul(xin, pred, MB[0:1, u_sl])
                nc.vector.tensor_add(xin, xin, S2[0:1, u_sl])

                # ---- PE: W_n (opener + 16), GIN opener ----
                mm(G_n, BC[:, 256:384], E4, True, False)
                for j in range(KC):
                    for k in range(KC):
                        mm(jsl(G_n, j), w_block(8 + j, k), jsl(h_cur, k),
                           False, k == KC - 1)
                mm(GIN, BC[:, 512:640], E4, True, False)

                # ---- PE: W_z opener + j01 ----
                mm(G_z, BC[:, 128:256], E4, True, False)
                for j in (0, 1):
                    for k in range(KC):
                        mm(jsl(G_z, j), w_block(4 + j, k), jsl(h_cur, k),
                           False, False)

                # ---- PE: XT (x_in broadcast to 4 partitions), X4, gi x3 ----
                for j in range(KC):
                    mm(XTp[:, j * bl:(j + 1) * bl], O14, xin, True, True)
                X4 = work.tile([4, KC * bl], f16, tag="X4")
                nc.vector.tensor_mul(X4, E4, XTp)

                mm(G_r, GI4[:, 0:128], X4, False, True)
                mm(GIN, GI4[:, 256:384], X4, False, True)

                # ---- PE: W_z j23, gi_z ----
                for j in (2, 3):
                    for k in range(KC):
                        mm(jsl(G_z, j), w_block(4 + j, k), jsl(h_cur, k),
                           False, False)
                mm(G_z, GI4[:, 128:256], X4, False, True)

                # ---- chains ----
                r_sb = work.tile([128, KC * bl], f16, tag="r_sb")
                nc.scalar.activation(out=r_sb, in_=G_r, func=Sigmoid)
                z_sb = work.tile([128, KC * bl], f16, tag="z_sb")
                nc.scalar.activation(out=z_sb, in_=G_z, func=Sigmoid)

                u_n = work.tile([128, KC * bl], f32, tag="u_n")
                nc.vector.tensor_mul(u_n, r_sb, G_n)
                nc.vector.tensor_add(PREN, u_n, GIN)

                n_sb = work.tile([128, KC * bl], f16, tag="n_sb")
                nc.scalar.activation(
                    out=n_sb[:, 0:HB], in_=PREN[:, 0:HB], func=Tanh)
                nc.scalar.activation(
                    out=n_sb[:, HB:2 * HB], in_=PREN[:, HB:2 * HB], func=Tanh)

                t1 = work.tile([128, KC * bl], f16, tag="t1")
                nc.vector.tensor_mul(t1, z_sb, h_cur)

                # h' = t1 - (z-1)*n, in two halves so next-step PE restarts early
                t2 = work.tile([128, KC * bl], f16, tag="t2")
                for lo, hi in ((0, HB), (HB, 2 * HB)):
                    nc.vector.scalar_tensor_tensor(
                        out=t2[:, lo:hi], in0=z_sb[:, lo:hi], scalar=1.0,
                        in1=n_sb[:, lo:hi], op0=SUB, op1=MULT,
                    )
                    nc.vector.tensor_sub(
                        h_new[:, lo:hi], t1[:, lo:hi], t2[:, lo:hi])

                # stream h' out:  outl[t, p, j, b]
                dst = d_out[bass.ds(t_dyn, 1)].rearrange("o p j b -> (o p) j b")
                nc.sync.dma_start(
                    out=dst, in_=h_new.rearrange("p (j b) -> p j b", b=bl)
                )

            n_blocks = t_steps // u_steps
            with tc.For_i(
                0, n_blocks, 1, hint_engines=(mybir.EngineType.PE,)
            ) as iv:
                S2 = io.tile([1, u_steps * bl], f16, tag="S2")
                MB = io.tile([1, u_steps * bl], f16, tag="MB")
                nc.sync.dma_start(
                    out=S2[0:1, :].rearrange("p (u b) -> p u b", b=bl),
                    in_=d_a[bass.ds(iv * u_steps, u_steps)].unsqueeze(0),
                )
                nc.sync.dma_start(
                    out=MB[0:1, :].rearrange("p (u b) -> p u b", b=bl),
                    in_=d_m[bass.ds(iv * u_steps, u_steps)].unsqueeze(0),
                )
                chain = [None]
                for u in range(u_steps):
                    step(
                        u,
                        h_tiles[u % 2],
                        h_tiles[(u + 1) % 2],
                        S2,
                        MB,
                        iv * u_steps + u,
                        chain,
                    )

    nc.compile()
    return nc


def _prep_core_inputs(inputs, core, t_steps=T, bl=BL):
    """Build the per-core input map (numpy) for core id `core`."""
    f16 = np.float16
    direction = 0 if core < 4 else 1  # 0 fwd, 1 bwd
    bg = core % 4
    sl = slice(bg * bl, (bg + 1) * bl)

    x = np.asarray(inputs["x"], np.float32)[:, :, 0]      # [B, T]
    msk = np.asarray(inputs["mask"]).astype(np.float32)[:, :, 0]
    pfx = "wf" if direction == 0 else "wb"
    w_ih = np.asarray(inputs[f"{pfx}_ih"], np.float32)[:, 0]   # [3H]
    w_hh = np.asarray(inputs[f"{pfx}_hh"], np.float32)         # [3H, H]
    b_ih = np.asarray(inputs[f"b{pfx[1]}_ih"], np.float32)
    b_hh = np.asarray(inputs[f"b{pfx[1]}_hh"], np.float32)
    p_w1 = np.asarray(inputs["p_w1"], np.float32)
    p_b1 = np.asarray(inputs["p_b1"], np.float32)
    p_w2 = np.asarray(inputs["p_w2"], np.float32)
    p_b2 = np.asarray(inputs["p_b2"], np.float32)

    xs = x[sl].T.copy()      # [T, bl]
    ms = msk[sl].T.copy()
    if direction == 1:
        xs = xs[::-1].copy()
        ms = ms[::-1].copy()
    a_arr = (xs * (1.0 - ms)).astype(f16)       # [T, bl]
    m_arr = ms.astype(f16)

    W = np.concatenate([w_hh, p_w1], axis=0)             # [2048, 512]
    Wr = W.reshape(MC, 128, KC, 128)                     # [m, c, k, p]
    wt = Wr.transpose(3, 0, 2, 1).reshape(128, MC * KC * 128).astype(f16)

    # consolidated gi stationaries: per gate g a [4,128] block (row j' =
    # w_ih chunk j'), contracted against X4 = blockdiag(x_in)
    gi4 = np.ascontiguousarray(
        w_ih.reshape(3, KC, 128).transpose(1, 0, 2).reshape(KC, 3 * 128)
    ).astype(f16)

    # v1-style bias columns: r, z, n(b_hh), ph(p_b1), gin(b_ih_n)
    bias_regions = [
        b_ih[0:H] + b_hh[0:H],          # r
        b_ih[H:2 * H] + b_hh[H:2 * H],  # z
        b_hh[2 * H:3 * H],              # n: b_hh only
        p_b1,                           # ph
        b_ih[2 * H:3 * H],              # gin: b_ih_n
    ]
    bc5 = np.concatenate(
        [br.reshape(KC, 128) for br in bias_regions], axis=1
    ).astype(f16)                                        # [4, 5*128]

    e4 = np.zeros((KC, KC, bl), np.float32)
    for j in range(KC):
        e4[j, j, :] = 1.0
    e4 = e4.reshape(KC, KC * bl).astype(f16)

    pw2t = p_w2[0].reshape(KC, 128).T.astype(f16).copy()
    pb2 = p_b2.reshape(1, 1).astype(np.float32)
    o14 = np.ones((1, 4), f16)

    return {
        "wt": wt, "gi4": gi4, "bc5": bc5, "o14": o14, "e4": e4,
        "pw2t": pw2t, "pb2": pb2,
        "a_arr": a_arr[:t_steps], "m_arr": m_arr[:t_steps],
    }


def _assemble(results, t_steps=T, bl=BL):
    """results: list of 8 per-core dicts with 'outl' [T,128,KC,bl] fp16."""
    out = np.zeros((B, t_steps, 2 * H), np.float32)
    for core in range(NCORES):
        direction = 0 if core < 4 else 1
        bg = core % 4
        arr = np.asarray(results[core]["outl"], np.float16).astype(np.float32)
        # [t, p, j, b] -> [b, t, j, p] -> [b, t, 512]
        arr = arr.transpose(3, 0, 2, 1).reshape(bl, t_steps, H)
        if direction == 1:
            arr = arr[:, ::-1]
        out[bg * bl:(bg + 1) * bl, :, direction * H:(direction + 1) * H] = arr
    return out


def kernel(**inputs):
    from concourse.bass_utils import run_bass_kernel_spmd

    key = (T, U_DEF, BL)
    if key not in _cache:
        _cache[key] = _build_program(T, U_DEF, BL)
    nc = _cache[key]

    in_maps = [_prep_core_inputs(inputs, c) for c in range(NCORES)]
    res = run_bass_kernel_spmd(
        nc, in_maps, core_ids=list(range(NCORES)), trace=False
    )
    return _assemble(res.results)


# revision 7
# speedup vs baseline: 1.2448x; 1.2448x over previous
# kernel.py — Bidirectional masked-GRU-with-predictor on 8 Trainium2 NeuronCores.
#
# Problem (reference.py): B=128, T=1024, H=512
#   per step, per direction:
#     x_in = where(mask, predictor(h), x)            predictor: Linear(H,H)->ReLU->Linear(H,1)->Tanh
#     h    = GRUCell(h, x_in)                        PyTorch gate order (r, z, n)
#   output [B, T, 2H] = concat(fwd hidden states, time-reversed bwd hidden states)
#
# Sharding: 8 cores = 2 directions x 4 batch groups of 32.  All cores run the
# SAME Bass program; per-core data differs (bwd cores get time-reversed x/mask
# and their outputs are flipped back on the host).
#
# v5 schedule notes (vs the v1 baseline, which this is structurally close to):
#   - Regions are opened v1-style (one whole-region E4 bias matmul with
#     start=True, everything else start=False) — per-j start=True accumulation
#     was measured to corrupt results.
#   - The predictor chain is pulled earlier: W_r is split around PRD so the
#     PRD matmuls run ~0.9us into the step, right as the (half-split) relu
#     completes; predm then lands before the PE reaches gi_r.
#   - GIN (opener + gi) is emitted right after gi_r so PREN = u_n + GIN is
#     never blocked on late PE work; W_z/gi_z follow, so sigma_z lands in the
#     ACT gap between sigma_r and tanh(n).
#   - tanh(n) and the final combine run in half-tiles, and PH is emitted with
#     a k01/k23 phase split, so the next step's PE stream restarts after only
#     the first half of h_new is written.
#   - u_steps=64 halves the For_i block-boundary cost.

import numpy as np

B, T, H = 128, 1024, 512
NCORES = 8
BL = B // 4          # 32: batch per core (4 groups x 2 directions)
KC = H // 128        # 4 contraction chunks
MC = (3 * H + H) // 128  # 16 output chunks (w_hh 12 + p_w1 4)
U_DEF = 64           # time steps per For_i iteration

_cache = {}


def _build_program(t_steps=T, u_steps=U_DEF, bl=BL, n_cores=NCORES):
    import concourse.bacc as bacc
    import concourse.bass as bass
    import concourse.tile as tile
    from concourse.tile import add_dep_helper
    from concourse import mybir

    f16 = mybir.dt.float16
    f32 = mybir.dt.float32

    nc = bacc.Bacc(
        "TRN2",
        target_bir_lowering=False,
        debug=False,
        enable_asserts=False,
        num_devices=n_cores,
    )

    # ---- DRAM tensors (per-core data; same names on every core) ----
    d_wt = nc.dram_tensor("wt", [128, MC * KC * 128], f16, kind="ExternalInput").ap()
    d_gi = nc.dram_tensor("gilhs", [2, 12 * 128], f16, kind="ExternalInput").ap()
    d_bc = nc.dram_tensor("bcols", [4, 5 * 128], f16, kind="ExternalInput").ap()
    d_e4 = nc.dram_tensor("e4", [4, KC * bl], f16, kind="ExternalInput").ap()
    d_pw2 = nc.dram_tensor("pw2t", [128, KC], f16, kind="ExternalInput").ap()
    d_pb2 = nc.dram_tensor("pb2", [1, 1], f32, kind="ExternalInput").ap()
    d_a = nc.dram_tensor("a_arr", [t_steps, bl], f16, kind="ExternalInput").ap()
    d_m = nc.dram_tensor("m_arr", [t_steps, bl], f16, kind="ExternalInput").ap()
    d_out = nc.dram_tensor(
        "outl", [t_steps, 128, KC, bl], f16, kind="ExternalOutput"
    ).ap()

    Tanh = mybir.ActivationFunctionType.Tanh
    Sigmoid = mybir.ActivationFunctionType.Sigmoid
    SUB = mybir.AluOpType.subtract
    MULT = mybir.AluOpType.mult

    HB = KC * bl // 2    # 64: half of the (j,b) free dim

    with tile.TileContext(nc) as tc:
        import contextlib

        with contextlib.ExitStack() as ctx:
            consts = ctx.enter_context(tc.tile_pool(name="consts", bufs=1))
            psum = ctx.enter_context(tc.tile_pool(name="psum", bufs=1, space="PSUM"))
            work = ctx.enter_context(tc.tile_pool(name="work", bufs=2))
            io = ctx.enter_context(tc.tile_pool(name="io", bufs=2))

            # ---- constant preload ----
            WT = consts.tile([128, MC * KC * 128], f16, tag="WT")
            GIL = consts.tile([2, 12 * 128], f16, tag="GIL")
            BC = consts.tile([4, 5 * 128], f16, tag="BC")
            E4 = consts.tile([4, KC * bl], f16, tag="E4")
            PW2 = consts.tile([128, KC], f16, tag="PW2")
            PB2 = consts.tile([1, 1], f32, tag="PB2")
            for dst, src in (
                (WT, d_wt), (GIL, d_gi), (BC, d_bc),
                (E4, d_e4), (PW2, d_pw2), (PB2, d_pb2),
            ):
                nc.sync.dma_start(out=dst, in_=src)

            # persistent ping-pong hidden state, fp16, [128, (j,b)]
            h0 = consts.tile([128, KC * bl], f16, tag="h0")
            h1 = consts.tile([128, KC * bl], f16, tag="h1")
            nc.vector.memset(h0, 0.0)
            nc.vector.memset(h1, 0.0)
            h_tiles = [h0, h1]

            # persistent PSUM accumulators (single-buffered; readers drain early)
            G_r = psum.tile([128, KC * bl], f32, tag="G_r")
            G_z = psum.tile([128, KC * bl], f32, tag="G_z")
            G_n = psum.tile([128, KC * bl], f32, tag="G_n")
            GIN = psum.tile([128, KC * bl], f32, tag="GIN")
            PHp = psum.tile([128, KC * bl], f32, tag="PH")
            PREN = psum.tile([128, KC * bl], f32, tag="PREN")
            PRD = psum.tile([1, bl], f32, tag="PRD")

            def w_block(m, k):
                bi = m * KC + k
                return WT[:, bi * 128:(bi + 1) * 128]

            def jsl(t, j):
                return t[:, j * bl:(j + 1) * bl]

            def step(u, h_cur, h_new, S2, MB, t_dyn, chain):
                # chain: [last PE instruction] single-element list used to
                # force PE issue order across regions/steps (ordering only).
                def mm(out, lhsT, rhs, start, stop):
                    ins = nc.tensor.matmul(
                        out, lhsT, rhs, start=start, stop=stop,
                        skip_group_check=True,
                    )
                    if chain[0] is not None:
                        add_dep_helper(ins.ins, chain[0].ins, sync=False)
                    chain[0] = ins
                    return ins

                u_sl = slice(u * bl, (u + 1) * bl)
                gi_rhs = S2[:, u_sl]

                def gi(region, g_idx, j, stop=True):
                    gj = g_idx * KC + j
                    return mm(jsl(region, j),
                              GIL[:, gj * 128:(gj + 1) * 128], gi_rhs,
                              False, stop)

                # ---- PE: PH region (opener + phase k01 / k23) ----
                mm(PHp, BC[:, 384:512], E4, True, False)
                for j in range(KC):
                    for k in (0, 1):
                        mm(jsl(PHp, j), w_block(12 + j, k), jsl(h_cur, k),
                           False, False)
                for j in range(KC):
                    for k in (2, 3):
                        mm(jsl(PHp, j), w_block(12 + j, k), jsl(h_cur, k),
                           False, k == 3)

                # relu in halves on DVE; PRD k01 needs only the first half
                relu = work.tile([128, KC * bl], f16, tag="relu")
                nc.vector.tensor_scalar_max(relu[:, 0:HB], PHp[:, 0:HB], 0.0)
                nc.vector.tensor_scalar_max(
                    relu[:, HB:2 * HB], PHp[:, HB:2 * HB], 0.0)

                # ---- PE: W_r j01, PRD, W_r j23 ----
                mm(G_r, BC[:, 0:128], E4, True, False)
                for j in (0, 1):
                    for k in range(KC):
                        mm(jsl(G_r, j), w_block(j, k), jsl(h_cur, k),
                           False, False)
                for k in range(KC):
                    mm(PRD, PW2[:, k:k + 1], jsl(relu, k), k == 0, k == KC - 1)
                for j in (2, 3):
                    for k in range(KC):
                        mm(jsl(G_r, j), w_block(j, k), jsl(h_cur, k),
                           False, False)

                pred = work.tile([1, bl], f16, tag="pred")
                nc.scalar.activation(out=pred, in_=PRD, func=Tanh, bias=PB2[:, :])
                nc.vector.tensor_mul(S2[0:1, u_sl], pred, MB[0:1, u_sl])

                # ---- PE: W_n (opener + 16) ----
                mm(G_n, BC[:, 256:384], E4, True, False)
                for j in range(KC):
                    for k in range(KC):
                        mm(jsl(G_n, j), w_block(8 + j, k), jsl(h_cur, k),
                           False, k == KC - 1)

                # ---- PE: gi_r, GIN (opener + gi) ----
                for j in range(KC):
                    gi(G_r, 0, j)
                mm(GIN, BC[:, 512:640], E4, True, False)
                for j in range(KC):
                    gi(GIN, 2, j)

                # ---- PE: W_z (opener + 16) + gi_z ----
                mm(G_z, BC[:, 128:256], E4, True, False)
                for j in range(KC):
                    for k in range(KC):
                        mm(jsl(G_z, j), w_block(4 + j, k), jsl(h_cur, k),
                           False, False)
                for j in range(KC):
                    gi(G_z, 1, j)

                # ---- chains ----
                r_sb = work.tile([128, KC * bl], f16, tag="r_sb")
                nc.scalar.activation(out=r_sb, in_=G_r, func=Sigmoid)
                z_sb = work.tile([128, KC * bl], f16, tag="z_sb")
                nc.scalar.activation(out=z_sb, in_=G_z, func=Sigmoid)

                u_n = work.tile([128, KC * bl], f32, tag="u_n")
                nc.vector.tensor_mul(u_n, r_sb, G_n)
                nc.vector.tensor_add(PREN, u_n, GIN)

                n_sb = work.tile([128, KC * bl], f16, tag="n_sb")
                nc.scalar.activation(
                    out=n_sb[:, 0:HB], in_=PREN[:, 0:HB], func=Tanh)
                nc.scalar.activation(
                    out=n_sb[:, HB:2 * HB], in_=PREN[:, HB:2 * HB], func=Tanh)

                t1 = work.tile([128, KC * bl], f16, tag="t1")
                nc.vector.tensor_mul(t1, z_sb, h_cur)

                # h' = t1 - (z-1)*n, in two halves so next-step PE restarts early
                t2 = work.tile([128, KC * bl], f16, tag="t2")
                for lo, hi in ((0, HB), (HB, 2 * HB)):
                    nc.vector.scalar_tensor_tensor(
                        out=t2[:, lo:hi], in0=z_sb[:, lo:hi], scalar=1.0,
                        in1=n_sb[:, lo:hi], op0=SUB, op1=MULT,
                    )
                    nc.vector.tensor_sub(
                        h_new[:, lo:hi], t1[:, lo:hi], t2[:, lo:hi])

                # stream h' out:  outl[t, p, j, b]
                dst = d_out[bass.ds(t_dyn, 1)].rearrange("o p j b -> (o p) j b")
                nc.sync.dma_start(
                    out=dst, in_=h_new.rearrange("p (j b) -> p j b", b=bl)
                )

            n_blocks = t_steps // u_steps
            with tc.For_i(
                0, n_blocks, 1, hint_engines=(mybir.EngineType.PE,)
            ) as iv:
                S2 = io.tile([2, u_steps * bl], f16, tag="S2")
                MB = io.tile([1, u_steps * bl], f16, tag="MB")
                nc.sync.dma_start(
                    out=S2[1:2, :].rearrange("p (u b) -> p u b", b=bl),
                    in_=d_a[bass.ds(iv * u_steps, u_steps)].unsqueeze(0),
                )
                nc.sync.dma_start(
                    out=MB[0:1, :].rearrange("p (u b) -> p u b", b=bl),
                    in_=d_m[bass.ds(iv * u_steps, u_steps)].unsqueeze(0),
                )
                chain = [None]
                for u in range(u_steps):
                    step(
                        u,
                        h_tiles[u % 2],
                        h_tiles[(u + 1) % 2],
                        S2,
                        MB,
                        iv * u_steps + u,
                        chain,
                    )

    nc.compile()
    return nc


def _prep_core_inputs(inputs, core, t_steps=T, bl=BL):
    """Build the per-core input map (numpy) for core id `core`."""
    f16 = np.float16
    direction = 0 if core < 4 else 1  # 0 fwd, 1 bwd
    bg = core % 4
    sl = slice(bg * bl, (bg + 1) * bl)

    x = np.asarray(inputs["x"], np.float32)[:, :, 0]      # [B, T]
    msk = np.asarray(inputs["mask"]).astype(np.float32)[:, :, 0]
    pfx = "wf" if direction == 0 else "wb"
    w_ih = np.asarray(inputs[f"{pfx}_ih"], np.float32)[:, 0]   # [3H]
    w_hh = np.asarray(inputs[f"{pfx}_hh"], np.float32)         # [3H, H]
    b_ih = np.asarray(inputs[f"b{pfx[1]}_ih"], np.float32)
    b_hh = np.asarray(inputs[f"b{pfx[1]}_hh"], np.float32)
    p_w1 = np.asarray(inputs["p_w1"], np.float32)
    p_b1 = np.asarray(inputs["p_b1"], np.float32)
    p_w2 = np.asarray(inputs["p_w2"], np.float32)
    p_b2 = np.asarray(inputs["p_b2"], np.float32)

    xs = x[sl].T.copy()      # [T, bl]
    ms = msk[sl].T.copy()
    if direction == 1:
        xs = xs[::-1].copy()
        ms = ms[::-1].copy()
    a_arr = (xs * (1.0 - ms)).astype(f16)
    m_arr = ms.astype(f16)

    W = np.concatenate([w_hh, p_w1], axis=0)             # [2048, 512]
    Wr = W.reshape(MC, 128, KC, 128)                     # [m, c, k, p]
    wt = Wr.transpose(3, 0, 2, 1).reshape(128, MC * KC * 128).astype(f16)

    # gi stationaries: per (gate g, chunk j) a [2,128] block, both rows =
    # w_ih[g*512 + j*128 : ...]; contract with [predm; a] rows of S2.
    gilhs = np.broadcast_to(
        w_ih.reshape(3 * KC, 128)[None, :, :], (2, 3 * KC, 128)
    ).reshape(2, 12 * 128).astype(f16).copy()

    bias_regions = [
        b_ih[0:H] + b_hh[0:H],          # r
        b_ih[H:2 * H] + b_hh[H:2 * H],  # z
        b_hh[2 * H:3 * H],              # n: b_hh only
        p_b1,                           # ph
        b_ih[2 * H:3 * H],              # gin: b_ih_n
    ]
    bcols = np.concatenate(
        [br.reshape(KC, 128) for br in bias_regions], axis=1
    ).astype(f16)                                        # [4, 5*128]

    e4 = np.zeros((KC, KC, bl), np.float32)
    for j in range(KC):
        e4[j, j, :] = 1.0
    e4 = e4.reshape(KC, KC * bl).astype(f16)

    pw2t = p_w2[0].reshape(KC, 128).T.astype(f16).copy()
    pb2 = p_b2.reshape(1, 1).astype(np.float32)

    return {
        "wt": wt, "gilhs": gilhs, "bcols": bcols, "e4": e4,
        "pw2t": pw2t, "pb2": pb2,
        "a_arr": a_arr[:t_steps], "m_arr": m_arr[:t_steps],
    }


def _assemble(results, t_steps=T, bl=BL):
    """results: list of 8 per-core dicts with 'outl' [T,128,KC,bl] fp16."""
    out = np.zeros((B, t_steps, 2 * H), np.float32)
    for core in range(NCORES):
        direction = 0 if core < 4 else 1
        bg = core % 4
        arr = np.asarray(results[core]["outl"], np.float16).astype(np.float32)
        # [t, p, j, b] -> [b, t, j, p] -> [b, t, 512]
        arr = arr.transpose(3, 0, 2, 1).reshape(bl, t_steps, H)
        if direction == 1:
            arr = arr[:, ::-1]
        out[bg * bl:(bg + 1) * bl, :, direction * H:(direction + 1) * H] = arr
    return out


def kernel(**inputs):
    from concourse.bass_utils import run_bass_kernel_spmd

    key = (T, U_DEF, BL)
    if key not in _cache:
        _cache[key] = _build_program(T, U_DEF, BL)
    nc = _cache[key]

    in_maps = [_prep_core_inputs(inputs, c) for c in range(NCORES)]
    res = run_bass_kernel_spmd(
        nc, in_maps, core_ids=list(range(NCORES)), trace=False
    )
    return _assemble(res.results)


# revision 8
# speedup vs baseline: 1.4838x; 1.1920x over previous
# kernel.py — Bidirectional masked-GRU-with-predictor on 8 Trainium2 NeuronCores.
#
# Problem (reference.py): B=128, T=1024, H=512
#   per step, per direction:
#     x_in = where(mask, predictor(h), x)            predictor: Linear(H,H)->ReLU->Linear(H,1)->Tanh
#     h    = GRUCell(h, x_in)                        PyTorch gate order (r, z, n)
#   output [B, T, 2H] = concat(fwd hidden states, time-reversed bwd hidden states)
#
# Sharding: 8 cores = 2 directions x 4 batch groups of 32.  All cores run the
# SAME Bass program; per-core data differs (bwd cores get time-reversed x/mask
# and their outputs are flipped back on the host).
#
# v5 schedule notes (vs the v1 baseline, which this is structurally close to):
#   - Regions are opened v1-style (one whole-region E4 bias matmul with
#     start=True, everything else start=False) — per-j start=True accumulation
#     was measured to corrupt results.
#   - The predictor chain is pulled earlier: W_r is split around PRD so the
#     PRD matmuls run ~0.9us into the step, right as the (half-split) relu
#     completes; predm then lands before the PE reaches gi_r.
#   - GIN (opener + gi) is emitted right after gi_r so PREN = u_n + GIN is
#     never blocked on late PE work; W_z/gi_z follow, so sigma_z lands in the
#     ACT gap between sigma_r and tanh(n).
#   - tanh(n) and the final combine run in half-tiles, and PH is emitted with
#     a k01/k23 phase split, so the next step's PE stream restarts after only
#     the first half of h_new is written.
#   - u_steps=64 halves the For_i block-boundary cost.

import numpy as np

B, T, H = 128, 1024, 512
NCORES = 8
BL = B // 4          # 32: batch per core (4 groups x 2 directions)
KC = H // 128        # 4 contraction chunks
MC = (3 * H + H) // 128  # 16 output chunks (w_hh 12 + p_w1 4)
U_DEF = 64           # time steps per For_i iteration

_cache = {}


def _build_program(t_steps=T, u_steps=U_DEF, bl=BL, n_cores=NCORES):
    import concourse.bacc as bacc
    import concourse.bass as bass
    import concourse.tile as tile
    from concourse.tile import add_dep_helper
    from concourse import mybir

    f16 = mybir.dt.float16
    f32 = mybir.dt.float32

    nc = bacc.Bacc(
        "TRN2",
        target_bir_lowering=False,
        debug=False,
        enable_asserts=False,
        num_devices=n_cores,
    )

    # ---- DRAM tensors (per-core data; same names on every core) ----
    d_wt = nc.dram_tensor("wt", [128, MC * KC * 128], f16, kind="ExternalInput").ap()
    d_gi = nc.dram_tensor("gilhs", [2, 12 * 128], f16, kind="ExternalInput").ap()
    d_bc = nc.dram_tensor("bcols", [4, 5 * 128], f16, kind="ExternalInput").ap()
    d_e4 = nc.dram_tensor("e4", [4, KC * bl], f16, kind="ExternalInput").ap()
    d_pw2 = nc.dram_tensor("pw2t", [128, KC], f16, kind="ExternalInput").ap()
    d_pb2 = nc.dram_tensor("pb2", [1, 1], f32, kind="ExternalInput").ap()
    d_a = nc.dram_tensor("a_arr", [t_steps, bl], f16, kind="ExternalInput").ap()
    d_m = nc.dram_tensor("m_arr", [t_steps, bl], f16, kind="ExternalInput").ap()
    d_out = nc.dram_tensor(
        "outl", [t_steps, 128, KC, bl], f16, kind="ExternalOutput"
    ).ap()

    Tanh = mybir.ActivationFunctionType.Tanh
    Sigmoid = mybir.ActivationFunctionType.Sigmoid
    SUB = mybir.AluOpType.subtract
    MULT = mybir.AluOpType.mult

    HB = KC * bl // 2    # 64: half of the (j,b) free dim

    with tile.TileContext(nc) as tc:
        import contextlib

        with contextlib.ExitStack() as ctx:
            consts = ctx.enter_context(tc.tile_pool(name="consts", bufs=1))
            psum = ctx.enter_context(tc.tile_pool(name="psum", bufs=1, space="PSUM"))
            work = ctx.enter_context(tc.tile_pool(name="work", bufs=2))
            io = ctx.enter_context(tc.tile_pool(name="io", bufs=2))

            # ---- constant preload ----
            WT = consts.tile([128, MC * KC * 128], f16, tag="WT")
            GIL = consts.tile([2, 12 * 128], f16, tag="GIL")
            BC = consts.tile([4, 5 * 128], f16, tag="BC")
            E4 = consts.tile([4, KC * bl], f16, tag="E4")
            PW2 = consts.tile([128, KC], f16, tag="PW2")
            PB2 = consts.tile([1, 1], f32, tag="PB2")
            for dst, src in (
                (WT, d_wt), (GIL, d_gi), (BC, d_bc),
                (E4, d_e4), (PW2, d_pw2), (PB2, d_pb2),
            ):
                nc.sync.dma_start(out=dst, in_=src)

            # persistent ping-pong hidden state, fp16, [128, (j,b)]
            h0 = consts.tile([128, KC * bl], f16, tag="h0")
            h1 = consts.tile([128, KC * bl], f16, tag="h1")
            nc.vector.memset(h0, 0.0)
            nc.vector.memset(h1, 0.0)
            h_tiles = [h0, h1]

            # persistent PSUM accumulators (single-buffered; readers drain early)
            G_r = psum.tile([128, KC * bl], f32, tag="G_r")
            G_z = psum.tile([128, KC * bl], f32, tag="G_z")
            G_n = psum.tile([128, KC * bl], f32, tag="G_n")
            GIN = psum.tile([128, KC * bl], f32, tag="GIN")
            PHp = psum.tile([128, KC * bl], f32, tag="PH")
            PREN = psum.tile([128, KC * bl], f32, tag="PREN")
            PRD = psum.tile([1, bl], f32, tag="PRD")

            def w_block(m, k):
                bi = m * KC + k
                return WT[:, bi * 128:(bi + 1) * 128]

            def jsl(t, j):
                return t[:, j * bl:(j + 1) * bl]

            def step(u, h_cur, h_new, S2, MB, t_dyn, chain):
                # chain[0]: last PE instruction of the previous region; PE
                # issue order is forced only at region boundaries (v1-style).
                regions = []

                def mm(out, lhsT, rhs, start, stop):
                    ins = nc.tensor.matmul(
                        out, lhsT, rhs, start=start, stop=stop,
                        skip_group_check=True,
                    )
                    regions.append(ins)
                    return ins

                def close_region():
                    # order this region's first MM after the previous
                    # region's last MM (scheduling hint, no semaphore)
                    if regions and chain[0] is not None:
                        add_dep_helper(regions[0].ins, chain[0].ins, sync=False)
                    if regions:
                        chain[0] = regions[-1]
                    regions.clear()

                u_sl = slice(u * bl, (u + 1) * bl)
                gi_rhs = S2[:, u_sl]

                def gi(region, g_idx, j, stop=True):
                    gj = g_idx * KC + j
                    return mm(jsl(region, j),
                              GIL[:, gj * 128:(gj + 1) * 128], gi_rhs,
                              False, stop)

                # ---- PE: all 5 bias openers up front (their WAR hazards
                # against the previous step's readers are long satisfied) ----
                mm(PHp, BC[:, 384:512], E4, True, False)
                mm(G_r, BC[:, 0:128], E4, True, False)
                mm(G_z, BC[:, 128:256], E4, True, False)
                mm(G_n, BC[:, 256:384], E4, True, False)
                mm(GIN, BC[:, 512:640], E4, True, False)
                close_region()

                # ---- PE: PH region, phase k01 / k23 ----
                for j in range(KC):
                    for k in (0, 1):
                        mm(jsl(PHp, j), w_block(12 + j, k), jsl(h_cur, k),
                           False, False)
                for j in range(KC):
                    for k in (2, 3):
                        mm(jsl(PHp, j), w_block(12 + j, k), jsl(h_cur, k),
                           False, k == 3)
                close_region()

                # relu in halves on DVE; PRD k01 needs only the first half
                relu = work.tile([128, KC * bl], f16, tag="relu")
                nc.vector.tensor_scalar_max(relu[:, 0:HB], PHp[:, 0:HB], 0.0)
                nc.vector.tensor_scalar_max(
                    relu[:, HB:2 * HB], PHp[:, HB:2 * HB], 0.0)

                # ---- PE: W_r j012, PRD, W_r j3 ----
                for j in (0, 1, 2):
                    for k in range(KC):
                        mm(jsl(G_r, j), w_block(j, k), jsl(h_cur, k),
                           False, False)
                close_region()
                for k in range(KC):
                    mm(PRD, PW2[:, k:k + 1], jsl(relu, k), k == 0, k == KC - 1)
                close_region()
                for j in (3,):
                    for k in range(KC):
                        mm(jsl(G_r, j), w_block(j, k), jsl(h_cur, k),
                           False, False)
                close_region()

                pred = work.tile([1, bl], f16, tag="pred")
                nc.scalar.activation(out=pred, in_=PRD, func=Tanh, bias=PB2[:, :])
                nc.vector.tensor_mul(S2[0:1, u_sl], pred, MB[0:1, u_sl])

                # ---- PE: W_n ----
                for j in range(KC):
                    for k in range(KC):
                        mm(jsl(G_n, j), w_block(8 + j, k), jsl(h_cur, k),
                           False, k == KC - 1)
                close_region()

                # ---- PE: gi_r + gi_n (one contiguous aux run) ----
                for j in range(KC):
                    gi(G_r, 0, j)
                for j in range(KC):
                    gi(GIN, 2, j)
                close_region()

                # ---- PE: W_z + gi_z ----
                for j in range(KC):
                    for k in range(KC):
                        mm(jsl(G_z, j), w_block(4 + j, k), jsl(h_cur, k),
                           False, False)
                close_region()
                for j in range(KC):
                    gi(G_z, 1, j)
                close_region()

                # ---- chains ----
                r_sb = work.tile([128, KC * bl], f16, tag="r_sb")
                nc.scalar.activation(out=r_sb, in_=G_r, func=Sigmoid)
                z_sb = work.tile([128, KC * bl], f16, tag="z_sb")
                nc.scalar.activation(out=z_sb, in_=G_z, func=Sigmoid)

                u_n = work.tile([128, KC * bl], f32, tag="u_n")
                nc.vector.tensor_mul(u_n, r_sb, G_n)
                nc.vector.tensor_add(PREN, u_n, GIN)

                n_sb = work.tile([128, KC * bl], f16, tag="n_sb")
                nc.scalar.activation(
                    out=n_sb[:, 0:HB], in_=PREN[:, 0:HB], func=Tanh)
                nc.scalar.activation(
                    out=n_sb[:, HB:2 * HB], in_=PREN[:, HB:2 * HB], func=Tanh)

                t1 = work.tile([128, KC * bl], f16, tag="t1")
                nc.vector.tensor_mul(t1, z_sb, h_cur)

                # h' = t1 - (z-1)*n, in two halves so next-step PE restarts early
                t2 = work.tile([128, KC * bl], f16, tag="t2")
                for lo, hi in ((0, HB), (HB, 2 * HB)):
                    nc.vector.scalar_tensor_tensor(
                        out=t2[:, lo:hi], in0=z_sb[:, lo:hi], scalar=1.0,
                        in1=n_sb[:, lo:hi], op0=SUB, op1=MULT,
                    )
                    nc.vector.tensor_sub(
                        h_new[:, lo:hi], t1[:, lo:hi], t2[:, lo:hi])

                # stream h' out:  outl[t, p, j, b]
                dst = d_out[bass.ds(t_dyn, 1)].rearrange("o p j b -> (o p) j b")
                nc.sync.dma_start(
                    out=dst, in_=h_new.rearrange("p (j b) -> p j b", b=bl)
                )

            n_blocks = t_steps // u_steps
            with tc.For_i(
                0, n_blocks, 1, hint_engines=(mybir.EngineType.PE,)
            ) as iv:
                S2 = io.tile([2, u_steps * bl], f16, tag="S2")
                MB = io.tile([1, u_steps * bl], f16, tag="MB")
                nc.sync.dma_start(
                    out=S2[1:2, :].rearrange("p (u b) -> p u b", b=bl),
                    in_=d_a[bass.ds(iv * u_steps, u_steps)].unsqueeze(0),
                )
                nc.sync.dma_start(
                    out=MB[0:1, :].rearrange("p (u b) -> p u b", b=bl),
                    in_=d_m[bass.ds(iv * u_steps, u_steps)].unsqueeze(0),
                )
                chain = [None]
                for u in range(u_steps):
                    step(
                        u,
                        h_tiles[u % 2],
                        h_tiles[(u + 1) % 2],
                        S2,
                        MB,
                        iv * u_steps + u,
                        chain,
                    )

    nc.compile()
    return nc


def _prep_core_inputs(inputs, core, t_steps=T, bl=BL):
    """Build the per-core input map (numpy) for core id `core`."""
    f16 = np.float16
    direction = 0 if core < 4 else 1  # 0 fwd, 1 bwd
    bg = core % 4
    sl = slice(bg * bl, (bg + 1) * bl)

    x = np.asarray(inputs["x"], np.float32)[:, :, 0]      # [B, T]
    msk = np.asarray(inputs["mask"]).astype(np.float32)[:, :, 0]
    pfx = "wf" if direction == 0 else "wb"
    w_ih = np.asarray(inputs[f"{pfx}_ih"], np.float32)[:, 0]   # [3H]
    w_hh = np.asarray(inputs[f"{pfx}_hh"], np.float32)         # [3H, H]
    b_ih = np.asarray(inputs[f"b{pfx[1]}_ih"], np.float32)
    b_hh = np.asarray(inputs[f"b{pfx[1]}_hh"], np.float32)
    p_w1 = np.asarray(inputs["p_w1"], np.float32)
    p_b1 = np.asarray(inputs["p_b1"], np.float32)
    p_w2 = np.asarray(inputs["p_w2"], np.float32)
    p_b2 = np.asarray(inputs["p_b2"], np.float32)

    xs = x[sl].T.copy()      # [T, bl]
    ms = msk[sl].T.copy()
    if direction == 1:
        xs = xs[::-1].copy()
        ms = ms[::-1].copy()
    a_arr = (xs * (1.0 - ms)).astype(f16)
    m_arr = ms.astype(f16)

    W = np.concatenate([w_hh, p_w1], axis=0)             # [2048, 512]
    Wr = W.reshape(MC, 128, KC, 128)                     # [m, c, k, p]
    wt = Wr.transpose(3, 0, 2, 1).reshape(128, MC * KC * 128).astype(f16)

    # gi stationaries: per (gate g, chunk j) a [2,128] block, both rows =
    # w_ih[g*512 + j*128 : ...]; contract with [predm; a] rows of S2.
    gilhs = np.broadcast_to(
        w_ih.reshape(3 * KC, 128)[None, :, :], (2, 3 * KC, 128)
    ).reshape(2, 12 * 128).astype(f16).copy()

    bias_regions = [
        b_ih[0:H] + b_hh[0:H],          # r
        b_ih[H:2 * H] + b_hh[H:2 * H],  # z
        b_hh[2 * H:3 * H],              # n: b_hh only
        p_b1,                           # ph
        b_ih[2 * H:3 * H],              # gin: b_ih_n
    ]
    bcols = np.concatenate(
        [br.reshape(KC, 128) for br in bias_regions], axis=1
    ).astype(f16)                                        # [4, 5*128]

    e4 = np.zeros((KC, KC, bl), np.float32)
    for j in range(KC):
        e4[j, j, :] = 1.0
    e4 = e4.reshape(KC, KC * bl).astype(f16)

    pw2t = p_w2[0].reshape(KC, 128).T.astype(f16).copy()
    pb2 = p_b2.reshape(1, 1).astype(np.float32)

    return {
        "wt": wt, "gilhs": gilhs, "bcols": bcols, "e4": e4,
        "pw2t": pw2t, "pb2": pb2,
        "a_arr": a_arr[:t_steps], "m_arr": m_arr[:t_steps],
    }


def _assemble(results, t_steps=T, bl=BL):
    """results: list of 8 per-core dicts with 'outl' [T,128,KC,bl] fp16."""
    out = np.zeros((B, t_steps, 2 * H), np.float32)
    for core in range(NCORES):
        direction = 0 if core < 4 else 1
        bg = core % 4
        arr = np.asarray(results[core]["outl"], np.float16).astype(np.float32)
        # [t, p, j, b] -> [b, t, j, p] -> [b, t, 512]
        arr = arr.transpose(3, 0, 2, 1).reshape(bl, t_steps, H)
        if direction == 1:
            arr = arr[:, ::-1]
        out[bg * bl:(bg + 1) * bl, :, direction * H:(direction + 1) * H] = arr
    return out


def kernel(**inputs):
    from concourse.bass_utils import run_bass_kernel_spmd

    key = (T, U_DEF, BL)
    if key not in _cache:
        _cache[key] = _build_program(T, U_DEF, BL)
    nc = _cache[key]

    in_maps = [_prep_core_inputs(inputs, c) for c in range(NCORES)]
    res = run_bass_kernel_spmd(
        nc, in_maps, core_ids=list(range(NCORES)), trace=False
    )
    return _assemble(res.results)


# revision 9
# speedup vs baseline: 1.5581x; 1.0501x over previous
# kernel.py — Bidirectional masked-GRU-with-predictor on 8 Trainium2 NeuronCores.
#
# Problem (reference.py): B=128, T=1024, H=512
#   per step, per direction:
#     x_in = where(mask, predictor(h), x)            predictor: Linear(H,H)->ReLU->Linear(H,1)->Tanh
#     h    = GRUCell(h, x_in)                        PyTorch gate order (r, z, n)
#   output [B, T, 2H] = concat(fwd hidden states, time-reversed bwd hidden states)
#
# Sharding: 8 cores = 2 directions x 4 batch groups of 32.  All cores run the
# SAME Bass program; per-core data differs (bwd cores get time-reversed x/mask
# and their outputs are flipped back on the host).
#
# v5 schedule notes (vs the v1 baseline, which this is structurally close to):
#   - Regions are opened v1-style (one whole-region E4 bias matmul with
#     start=True, everything else start=False) — per-j start=True accumulation
#     was measured to corrupt results.
#   - The predictor chain is pulled earlier: W_r is split around PRD so the
#     PRD matmuls run ~0.9us into the step, right as the (half-split) relu
#     completes; predm then lands before the PE reaches gi_r.
#   - GIN (opener + gi) is emitted right after gi_r so PREN = u_n + GIN is
#     never blocked on late PE work; W_z/gi_z follow, so sigma_z lands in the
#     ACT gap between sigma_r and tanh(n).
#   - tanh(n) and the final combine run in half-tiles, and PH is emitted with
#     a k01/k23 phase split, so the next step's PE stream restarts after only
#     the first half of h_new is written.
#   - u_steps=64 halves the For_i block-boundary cost.

import numpy as np

B, T, H = 128, 1024, 512
NCORES = 8
BL = B // 4          # 32: batch per core (4 groups x 2 directions)
KC = H // 128        # 4 contraction chunks
MC = (3 * H + H) // 128  # 16 output chunks (w_hh 12 + p_w1 4)
U_DEF = 64           # time steps per For_i iteration

_cache = {}


def _build_program(t_steps=T, u_steps=U_DEF, bl=BL, n_cores=NCORES):
    import concourse.bacc as bacc
    import concourse.bass as bass
    import concourse.tile as tile
    from concourse.tile import add_dep_helper
    from concourse import mybir

    f16 = mybir.dt.float16
    f32 = mybir.dt.float32

    nc = bacc.Bacc(
        "TRN2",
        target_bir_lowering=False,
        debug=False,
        enable_asserts=False,
        num_devices=n_cores,
    )

    # ---- DRAM tensors (per-core data; same names on every core) ----
    d_wt = nc.dram_tensor("wt", [128, MC * KC * 128], f16, kind="ExternalInput").ap()
    d_gi = nc.dram_tensor("gilhs", [2, 12 * 128], f16, kind="ExternalInput").ap()
    d_bc = nc.dram_tensor("bcols", [4, 5 * 128], f16, kind="ExternalInput").ap()
    d_e4 = nc.dram_tensor("e4", [4, KC * bl], f16, kind="ExternalInput").ap()
    d_pw2 = nc.dram_tensor("pw2t", [128, KC], f16, kind="ExternalInput").ap()
    d_pb2 = nc.dram_tensor("pb2", [1, 1], f32, kind="ExternalInput").ap()
    d_a = nc.dram_tensor("a_arr", [t_steps, bl], f16, kind="ExternalInput").ap()
    d_m = nc.dram_tensor("m_arr", [t_steps, bl], f16, kind="ExternalInput").ap()
    d_out = nc.dram_tensor(
        "outl", [t_steps, 128, KC, bl], f16, kind="ExternalOutput"
    ).ap()

    Tanh = mybir.ActivationFunctionType.Tanh
    Sigmoid = mybir.ActivationFunctionType.Sigmoid
    SUB = mybir.AluOpType.subtract
    MULT = mybir.AluOpType.mult

    HB = KC * bl // 2    # 64: half of the (j,b) free dim

    with tile.TileContext(nc) as tc:
        import contextlib

        with contextlib.ExitStack() as ctx:
            consts = ctx.enter_context(tc.tile_pool(name="consts", bufs=1))
            psum = ctx.enter_context(tc.tile_pool(name="psum", bufs=1, space="PSUM"))
            work = ctx.enter_context(tc.tile_pool(name="work", bufs=2))
            io = ctx.enter_context(tc.tile_pool(name="io", bufs=2))

            # ---- constant preload ----
            WT = consts.tile([128, MC * KC * 128], f16, tag="WT")
            GIL = consts.tile([2, 12 * 128], f16, tag="GIL")
            BC = consts.tile([4, 5 * 128], f16, tag="BC")
            E4 = consts.tile([4, KC * bl], f16, tag="E4")
            PW2 = consts.tile([128, KC], f16, tag="PW2")
            PB2 = consts.tile([1, 1], f32, tag="PB2")
            for dst, src in (
                (WT, d_wt), (GIL, d_gi), (BC, d_bc),
                (E4, d_e4), (PW2, d_pw2), (PB2, d_pb2),
            ):
                nc.sync.dma_start(out=dst, in_=src)

            # persistent ping-pong hidden state, fp16, [128, (j,b)]
            h0 = consts.tile([128, KC * bl], f16, tag="h0")
            h1 = consts.tile([128, KC * bl], f16, tag="h1")
            nc.vector.memset(h0, 0.0)
            nc.vector.memset(h1, 0.0)
            h_tiles = [h0, h1]

            # persistent PSUM accumulators (single-buffered; readers drain early)
            G_r = psum.tile([128, KC * bl], f32, tag="G_r")
            G_z = psum.tile([128, KC * bl], f32, tag="G_z")
            G_n = psum.tile([128, KC * bl], f32, tag="G_n")
            GIN = psum.tile([128, KC * bl], f32, tag="GIN")
            PHp = psum.tile([128, KC * bl], f32, tag="PH")
            PREN = psum.tile([128, KC * bl], f32, tag="PREN")
            PRD = psum.tile([1, bl], f32, tag="PRD")

            def w_block(m, k):
                bi = m * KC + k
                return WT[:, bi * 128:(bi + 1) * 128]

            def jsl(t, j):
                return t[:, j * bl:(j + 1) * bl]

            def step(u, h_cur, h_new, S2, MB, t_dyn, chain):
                # chain[0]: last PE instruction of the previous region; PE
                # issue order is forced only at region boundaries (v1-style).
                regions = []

                def mm(out, lhsT, rhs, start, stop):
                    ins = nc.tensor.matmul(
                        out, lhsT, rhs, start=start, stop=stop,
                        skip_group_check=True,
                    )
                    regions.append(ins)
                    return ins

                def close_region():
                    # order this region's first MM after the previous
                    # region's last MM (scheduling hint, no semaphore)
                    if regions and chain[0] is not None:
                        add_dep_helper(regions[0].ins, chain[0].ins, sync=False)
                    if regions:
                        chain[0] = regions[-1]
                    regions.clear()

                u_sl = slice(u * bl, (u + 1) * bl)
                gi_rhs = S2[:, u_sl]

                def gi(region, g_idx, j, stop=True):
                    gj = g_idx * KC + j
                    return mm(jsl(region, j),
                              GIL[:, gj * 128:(gj + 1) * 128], gi_rhs,
                              False, stop)

                # ---- PE: PH/G_r/G_z openers up front (their WAR hazards —
                # relu/sigma_r/sigma_z of the previous step — are satisfied by
                # the time the tail ends).  G_n/GIN openers WAR against the
                # previous step's u_n/PREN reads, which end LAST in the chain;
                # front-loading them blocks the in-order PE, so they are
                # emitted later in the stream. ----
                mm(PHp, BC[:, 384:512], E4, True, False)
                mm(G_r, BC[:, 0:128], E4, True, False)
                mm(G_z, BC[:, 128:256], E4, True, False)
                close_region()

                # ---- PE: PH region, phase k01 / k23 ----
                for j in range(KC):
                    for k in (0, 1):
                        mm(jsl(PHp, j), w_block(12 + j, k), jsl(h_cur, k),
                           False, False)
                for j in range(KC):
                    for k in (2, 3):
                        mm(jsl(PHp, j), w_block(12 + j, k), jsl(h_cur, k),
                           False, k == 3)
                close_region()

                # relu in halves on DVE; PRD k01 needs only the first half
                relu = work.tile([128, KC * bl], f16, tag="relu")
                nc.vector.tensor_scalar_max(relu[:, 0:HB], PHp[:, 0:HB], 0.0)
                nc.vector.tensor_scalar_max(
                    relu[:, HB:2 * HB], PHp[:, HB:2 * HB], 0.0)

                # ---- PE: W_r j012, PRD, W_r j3 ----
                for j in (0, 1, 2):
                    for k in range(KC):
                        mm(jsl(G_r, j), w_block(j, k), jsl(h_cur, k),
                           False, False)
                close_region()
                for k in range(KC):
                    mm(PRD, PW2[:, k:k + 1], jsl(relu, k), k == 0, k == KC - 1)
                close_region()
                for j in (3,):
                    for k in range(KC):
                        mm(jsl(G_r, j), w_block(j, k), jsl(h_cur, k),
                           False, False)
                close_region()

                pred = work.tile([1, bl], f16, tag="pred")
                nc.scalar.activation(out=pred, in_=PRD, func=Tanh, bias=PB2[:, :])
                nc.vector.tensor_mul(S2[0:1, u_sl], pred, MB[0:1, u_sl])

                # ---- PE: W_n (opener here: prev-step u_n read long done) ----
                mm(G_n, BC[:, 256:384], E4, True, False)
                for j in range(KC):
                    for k in range(KC):
                        mm(jsl(G_n, j), w_block(8 + j, k), jsl(h_cur, k),
                           False, k == KC - 1)
                close_region()

                # ---- PE: gi_r + GIN opener + gi_n (one contiguous aux run) ----
                for j in range(KC):
                    gi(G_r, 0, j)
                mm(GIN, BC[:, 512:640], E4, True, False)
                for j in range(KC):
                    gi(GIN, 2, j)
                close_region()

                # ---- PE: W_z + gi_z ----
                for j in range(KC):
                    for k in range(KC):
                        mm(jsl(G_z, j), w_block(4 + j, k), jsl(h_cur, k),
                           False, False)
                close_region()
                for j in range(KC):
                    gi(G_z, 1, j)
                close_region()

                # ---- chains ----
                r_sb = work.tile([128, KC * bl], f16, tag="r_sb")
                nc.scalar.activation(out=r_sb, in_=G_r, func=Sigmoid)
                z_sb = work.tile([128, KC * bl], f16, tag="z_sb")
                nc.scalar.activation(out=z_sb, in_=G_z, func=Sigmoid)

                u_n = work.tile([128, KC * bl], f32, tag="u_n")
                nc.vector.tensor_mul(u_n, r_sb, G_n)
                ins_pren = nc.vector.tensor_add(PREN, u_n, GIN)

                n_sb = work.tile([128, KC * bl], f16, tag="n_sb")
                nc.scalar.activation(
                    out=n_sb[:, 0:HB], in_=PREN[:, 0:HB], func=Tanh)
                nc.scalar.activation(
                    out=n_sb[:, HB:2 * HB], in_=PREN[:, HB:2 * HB], func=Tanh)

                # force DVE order t1-after-PREN: PREN gates tanh(n), the
                # chain-critical path; t1 only gates the final combine
                t1 = work.tile([128, KC * bl], f16, tag="t1")
                ins_t1 = nc.vector.tensor_mul(t1, z_sb, h_cur)
                add_dep_helper(ins_t1.ins, ins_pren.ins, sync=False)

                # h' = t1 - (z-1)*n, in two halves so next-step PE restarts early
                t2 = work.tile([128, KC * bl], f16, tag="t2")
                for lo, hi in ((0, HB), (HB, 2 * HB)):
                    nc.vector.scalar_tensor_tensor(
                        out=t2[:, lo:hi], in0=z_sb[:, lo:hi], scalar=1.0,
                        in1=n_sb[:, lo:hi], op0=SUB, op1=MULT,
                    )
                    nc.vector.tensor_sub(
                        h_new[:, lo:hi], t1[:, lo:hi], t2[:, lo:hi])

                # stream h' out:  outl[t, p, j, b]
                dst = d_out[bass.ds(t_dyn, 1)].rearrange("o p j b -> (o p) j b")
                nc.sync.dma_start(
                    out=dst, in_=h_new.rearrange("p (j b) -> p j b", b=bl)
                )

            n_blocks = t_steps // u_steps
            with tc.For_i(
                0, n_blocks, 1, hint_engines=(mybir.EngineType.PE,)
            ) as iv:
                S2 = io.tile([2, u_steps * bl], f16, tag="S2")
                MB = io.tile([1, u_steps * bl], f16, tag="MB")
                nc.sync.dma_start(
                    out=S2[1:2, :].rearrange("p (u b) -> p u b", b=bl),
                    in_=d_a[bass.ds(iv * u_steps, u_steps)].unsqueeze(0),
                )
                nc.sync.dma_start(
                    out=MB[0:1, :].rearrange("p (u b) -> p u b", b=bl),
                    in_=d_m[bass.ds(iv * u_steps, u_steps)].unsqueeze(0),
                )
                chain = [None]
                for u in range(u_steps):
                    step(
                        u,
                        h_tiles[u % 2],
                        h_tiles[(u + 1) % 2],
                        S2,
                        MB,
                        iv * u_steps + u,
                        chain,
                    )

    nc.compile()
    return nc


def _prep_core_inputs(inputs, core, t_steps=T, bl=BL):
    """Build the per-core input map (numpy) for core id `core`."""
    f16 = np.float16
    direction = 0 if core < 4 else 1  # 0 fwd, 1 bwd
    bg = core % 4
    sl = slice(bg * bl, (bg + 1) * bl)

    x = np.asarray(inputs["x"], np.float32)[:, :, 0]      # [B, T]
    msk = np.asarray(inputs["mask"]).astype(np.float32)[:, :, 0]
    pfx = "wf" if direction == 0 else "wb"
    w_ih = np.asarray(inputs[f"{pfx}_ih"], np.float32)[:, 0]   # [3H]
    w_hh = np.asarray(inputs[f"{pfx}_hh"], np.float32)         # [3H, H]
    b_ih = np.asarray(inputs[f"b{pfx[1]}_ih"], np.float32)
    b_hh = np.asarray(inputs[f"b{pfx[1]}_hh"], np.float32)
    p_w1 = np.asarray(inputs["p_w1"], np.float32)
    p_b1 = np.asarray(inputs["p_b1"], np.float32)
    p_w2 = np.asarray(inputs["p_w2"], np.float32)
    p_b2 = np.asarray(inputs["p_b2"], np.float32)

    xs = x[sl].T.copy()      # [T, bl]
    ms = msk[sl].T.copy()
    if direction == 1:
        xs = xs[::-1].copy()
        ms = ms[::-1].copy()
    a_arr = (xs * (1.0 - ms)).astype(f16)
    m_arr = ms.astype(f16)

    W = np.concatenate([w_hh, p_w1], axis=0)             # [2048, 512]
    Wr = W.reshape(MC, 128, KC, 128)                     # [m, c, k, p]
    wt = Wr.transpose(3, 0, 2, 1).reshape(128, MC * KC * 128).astype(f16)

    # gi stationaries: per (gate g, chunk j) a [2,128] block, both rows =
    # w_ih[g*512 + j*128 : ...]; contract with [predm; a] rows of S2.
    gilhs = np.broadcast_to(
        w_ih.reshape(3 * KC, 128)[None, :, :], (2, 3 * KC, 128)
    ).reshape(2, 12 * 128).astype(f16).copy()

    bias_regions = [
        b_ih[0:H] + b_hh[0:H],          # r
        b_ih[H:2 * H] + b_hh[H:2 * H],  # z
        b_hh[2 * H:3 * H],              # n: b_hh only
        p_b1,                           # ph
        b_ih[2 * H:3 * H],              # gin: b_ih_n
    ]
    bcols = np.concatenate(
        [br.reshape(KC, 128) for br in bias_regions], axis=1
    ).astype(f16)                                        # [4, 5*128]

    e4 = np.zeros((KC, KC, bl), np.float32)
    for j in range(KC):
        e4[j, j, :] = 1.0
    e4 = e4.reshape(KC, KC * bl).astype(f16)

    pw2t = p_w2[0].reshape(KC, 128).T.astype(f16).copy()
    pb2 = p_b2.reshape(1, 1).astype(np.float32)

    return {
        "wt": wt, "gilhs": gilhs, "bcols": bcols, "e4": e4,
        "pw2t": pw2t, "pb2": pb2,
        "a_arr": a_arr[:t_steps], "m_arr": m_arr[:t_steps],
    }


def _assemble(results, t_steps=T, bl=BL):
    """results: list of 8 per-core dicts with 'outl' [T,128,KC,bl] fp16."""
    out = np.zeros((B, t_steps, 2 * H), np.float32)
    for core in range(NCORES):
        direction = 0 if core < 4 else 1
        bg = core % 4
        arr = np.asarray(results[core]["outl"], np.float16).astype(np.float32)
        # [t, p, j, b] -> [b, t, j, p] -> [b, t, 512]
        arr = arr.transpose(3, 0, 2, 1).reshape(bl, t_steps, H)
        if direction == 1:
            arr = arr[:, ::-1]
        out[bg * bl:(bg + 1) * bl, :, direction * H:(direction + 1) * H] = arr
    return out


def kernel(**inputs):
    from concourse.bass_utils import run_bass_kernel_spmd

    key = (T, U_DEF, BL)
    if key not in _cache:
        _cache[key] = _build_program(T, U_DEF, BL)
    nc = _cache[key]

    in_maps = [_prep_core_inputs(inputs, c) for c in range(NCORES)]
    res = run_bass_kernel_spmd(
        nc, in_maps, core_ids=list(range(NCORES)), trace=False
    )
    return _assemble(res.results)
